# revision 58
# baseline (speedup 1.0000x reference)
"""Centroid triplet loss on 8 Trainium2 NeuronCores (Bass/Tile).

Class-sharded data parallel: the host assigns core k ALL samples whose label
falls in [32k, 32k+32) (padded to a fixed capacity with zero rows + an
out-of-range label).  Per-class embedding sums are then fully core-local, so
the only centroid communication is a 32KB bf16 AllGather of each core's 32
normalized centroid rows (vs. a 514KB AllReduce for unsorted sharding).

Math (equivalent to the reference):
    term_i = relu(margin + r_i * e_i . (cent[near(l_i)] - cent[l_i]))
    loss   = sum_c (1/count_c) * sum_{i in c} term_i / n_present
Per-sample gathers are replaced by matmuls against the 32-class one-hot:
pass 2 computes u_i = onehot_i . U (U = cent_near - cent_own, [32,512]) with
one PE matmul per 128-sample tile, dots it with e_i on DVE, and reduces the
per-class term sums S_c with tiny [128,1]x[128,32] matmuls.  Label-derived
scalars (1/count, presence mask, 1/n_present) are host-computed inputs.
"""

import sys

for _p in ("/opt/trn_rl_repo",):
    if _p not in sys.path:
        sys.path.insert(0, _p)

from contextlib import ExitStack

import ml_dtypes
import numpy as np

from concourse import bacc, bass, mybir, tile
from concourse.bass_utils import run_bass_kernel_spmd
from concourse.masks import make_identity

F32 = mybir.dt.float32
BF16 = mybir.dt.bfloat16
I32 = mybir.dt.int32
ALU = mybir.AluOpType
ACTF = mybir.ActivationFunctionType
AX = mybir.AxisListType.X

N_CORES = 8
B_FULL = 65536
D = 512
C = 256
C_LOC = C // N_CORES        # 32 classes owned per core
MARGIN = 0.3
EPS = 1e-12
NEG = -1e30

P = 128                      # SBUF partitions
B_CAP = 8704                 # padded per-core sample capacity (mean 8192)
T = B_CAP // P               # 68 sample tiles of 128
NCHUNK = 17                  # tiles per norm batch / load DMA
WARMUP_AR = True             # dummy tiny AllReduce to absorb CC bootstrap


def _build():
    nc = bacc.Bacc(
        "TRN2",
        target_bir_lowering=False,
        debug=False,
        enable_asserts=False,
        num_devices=N_CORES,
    )

    emb = nc.dram_tensor("emb", [B_CAP, D], BF16, kind="ExternalInput")
    lab = nc.dram_tensor("lab", [P, T], I32, kind="ExternalInput")
    ohtT_d = nc.dram_tensor("ohtT", [C_LOC, B_CAP], BF16, kind="ExternalInput")
    embT_d = nc.dram_tensor("embT", [D, B_CAP], BF16, kind="ExternalInput")
    negmask = nc.dram_tensor("negmask", [C_LOC, C], F32, kind="ExternalInput")
    wsamp = nc.dram_tensor("wsamp", [P, T], F32, kind="ExternalInput")
    loss_out = nc.dram_tensor("loss", [1, 1], F32, kind="ExternalOutput")

    ag_in = nc.dram_tensor("ag_in", [C_LOC, D], BF16)
    ag_out = nc.dram_tensor("ag_out", [C, D], BF16, addr_space="Shared")
    ar2_in = nc.dram_tensor("ar2_in", [1, 8], F32)
    ar2_out = nc.dram_tensor("ar2_out", [1, 8], F32, addr_space="Shared")
    if WARMUP_AR:
        ar0_in = nc.dram_tensor("ar0_in", [1, 8], F32)
        ar0_out = nc.dram_tensor("ar0_out", [1, 8], F32, addr_space="Shared")

    groups = [list(range(N_CORES))]

    with tile.TileContext(nc) as tc, ExitStack() as ctx:
        const = ctx.enter_context(tc.tile_pool(name="const", bufs=1))
        big = ctx.enter_context(tc.tile_pool(name="big", bufs=1))
        work = ctx.enter_context(tc.tile_pool(name="work", bufs=5))
        sq = ctx.enter_context(tc.tile_pool(name="sq", bufs=3))
        mid = ctx.enter_context(tc.tile_pool(name="mid", bufs=1))
        psacc = ctx.enter_context(tc.tile_pool(name="psacc", bufs=1, space="PSUM"))
        psmid = ctx.enter_context(tc.tile_pool(name="psmid", bufs=3, space="PSUM"))
        psu = ctx.enter_context(tc.tile_pool(name="psu", bufs=3, space="PSUM"))

        # ---- warm up the collective stream under the load DMA ----------
        if WARMUP_AR:
            ar0_sb = mid.tile([1, 8], F32, tag="ar0")
            nc.vector.memset(ar0_sb[:], 0.0)
            nc.sync.dma_start(out=ar0_in.ap()[:], in_=ar0_sb[:])
            nc.gpsimd.collective_compute(
                "AllReduce", ALU.add, replica_groups=groups,
                ins=[ar0_in.ap()], outs=[ar0_out.ap()],
            )

        # ---- constants -------------------------------------------------
        ident = const.tile([P, P], F32)
        make_identity(nc, ident[:])
        identb = const.tile([P, P], BF16)
        nc.vector.tensor_copy(out=identb[:], in_=ident[:])
        iota_row = const.tile([P, C_LOC], BF16)
        nc.gpsimd.iota(
            iota_row[:], pattern=[[1, C_LOC]], base=0, channel_multiplier=0,
            allow_small_or_imprecise_dtypes=True,
        )
        ones_col = const.tile([P, 1], F32)
        nc.gpsimd.memset(ones_col[:], 1.0)
        ones32 = const.tile([C_LOC, 1], BF16)
        nc.gpsimd.memset(ones32[:], 1.0)

        lab_sb = const.tile([P, T], I32)
        nc.sync.dma_start(out=lab_sb[:], in_=lab.ap())
        lab_f = const.tile([P, T], BF16)
        nc.vector.tensor_copy(out=lab_f[:], in_=lab_sb[:])
        # ---- pass 1: load embeddings, norms, local class sums ----------
        chunks = []
        t0 = 0
        while t0 < T:
            chunks.append((t0, min(NCHUNK, T - t0)))
            t0 += NCHUNK

        e_chunks = {}
        emb_v = emb.ap().rearrange("(t p) d -> p t d", p=P)
        for (c0, cn) in chunks:
            ec = big.tile([P, cn, D], BF16, tag=f"e{c0}")
            e_chunks[c0] = ec
            nc.sync.dma_start(out=ec[:], in_=emb_v[:, c0 : c0 + cn, :])

        # pass-2-only data loads go behind the embedding chunks
        nm_sb = const.tile([C_LOC, C], F32)
        nc.sync.dma_start(out=nm_sb[:], in_=negmask.ap())
        ws_sb = const.tile([P, T], F32)
        nc.sync.dma_start(out=ws_sb[:], in_=wsamp.ap())
        ohtT = const.tile([C_LOC, T * P], BF16)
        nc.sync.dma_start(out=ohtT[:], in_=ohtT_d.ap())
        eT = [
            const.tile([P, B_CAP], BF16, tag=f"eT{q}", name=f"eT{q}")
            for q in range(4)
        ]
        for q in range(4):
            nc.sync.dma_start(out=eT[q][:], in_=embT_d.ap()[q * P : (q + 1) * P, :])

        def e_tile(t):
            c0 = (t // NCHUNK) * NCHUNK
            return e_chunks[c0][:, t - c0, :]

        norm2 = const.tile([P, T], F32)
        r_all = const.tile([P, T], F32)
        r_bf = const.tile([P, T], BF16)

        sums_ps = psacc.tile([C_LOC, D], F32, tag="sums")

        for (c0, cn) in chunks:
            csl = slice(c0, c0 + cn)
            for j in range(cn):
                t = c0 + j
                et = e_tile(t)
                if t % 3 != 2:
                    sq_t = sq.tile([P, D], F32, tag="sq")
                    nc.scalar.activation(
                        sq_t[:], et, ACTF.Square, accum_out=norm2[:, t : t + 1]
                    )
                else:
                    pr_t = sq.tile([P, D], BF16, tag="pr")
                    nc.gpsimd.tensor_tensor(out=pr_t[:], in0=et, in1=et, op=ALU.mult)
                    nc.vector.reduce_sum(norm2[:, t : t + 1], pr_t[:], axis=AX)
            # batched norm -> r for the chunk (clamped so zero pads stay finite)
            nc.scalar.activation(r_all[:, csl], norm2[:, csl], ACTF.Sqrt)
            nc.vector.tensor_scalar(
                out=r_all[:, csl], in0=r_all[:, csl], scalar1=EPS, scalar2=None,
                op0=ALU.max,
            )
            nc.vector.reciprocal(r_all[:, csl], r_all[:, csl])
            nc.vector.tensor_copy(out=r_bf[:, csl], in_=r_all[:, csl])

            for j in range(cn):
                t = c0 + j
                oht_t = work.tile([P, C_LOC], BF16, tag="oht")
                nc.vector.tensor_tensor(
                    out=oht_t[:], in0=iota_row[:],
                    in1=lab_f[:, t : t + 1].to_broadcast([P, C_LOC]),
                    op=ALU.is_equal,
                )
                osc = work.tile([P, C_LOC], BF16, tag="osc")
                nc.vector.tensor_tensor(
                    out=osc[:], in0=oht_t[:],
                    in1=r_bf[:, t : t + 1].to_broadcast([P, C_LOC]),
                    op=ALU.mult,
                )
                nc.tensor.matmul(
                    sums_ps[:], osc[:], e_tile(t),
                    start=(t == 0), stop=(t == T - 1),
                )

        # ---- local centroids + AllGather -------------------------------
        sums_sb = mid.tile([C_LOC, D], F32, tag="ssb")
        nc.vector.tensor_copy(out=sums_sb[:], in_=sums_ps[:])
        s2 = sq.tile([C_LOC, D], F32, tag="sq")
        cn2 = mid.tile([C_LOC, 1], F32, tag="cn2")
        nc.scalar.activation(s2[:], sums_ps[:], ACTF.Square, accum_out=cn2[:])
        nc.scalar.activation(cn2[:], cn2[:], ACTF.Sqrt)
        nc.vector.tensor_scalar(
            out=cn2[:], in0=cn2[:], scalar1=EPS, scalar2=None, op0=ALU.max
        )
        nc.vector.reciprocal(cn2[:], cn2[:])
        cent_bf = mid.tile([C_LOC, D], BF16, tag="centbf")
        nc.vector.tensor_scalar(
            out=cent_bf[:], in0=sums_sb[:], scalar1=cn2[:], scalar2=None,
            op0=ALU.mult,
        )
        nc.sync.dma_start(out=ag_in.ap()[:], in_=cent_bf[:])
        nc.gpsimd.collective_compute(
            "AllGather", ALU.bypass, replica_groups=groups,
            ins=[ag_in.ap()], outs=[ag_out.ap()],
        )

        # local centroid transpose (f32 -> bf16 chunks) for the G matmul
        clT = [mid.tile([P, C_LOC], BF16, tag=f"clT{i}", name=f"clT{i}") for i in range(4)]
        for i in range(4):
            tp = psmid.tile([P, C_LOC], BF16, tag="m")
            nc.tensor.transpose(
                tp[:], cent_bf[:, i * P : (i + 1) * P], identb[0:C_LOC, 0:C_LOC]
            )
            nc.vector.tensor_copy(out=clT[i][:], in_=tp[:])

        # ---- gathered centroids; G rows; nearest; U --------------------
        cent_all = [mid.tile([P, D], BF16, tag=f"ca{h}", name=f"ca{h}") for h in range(2)]
        for h in range(2):
            nc.sync.dma_start(out=cent_all[h][:], in_=ag_out.ap()[h * P : (h + 1) * P, :])
        centT = [mid.tile([P, C], BF16, tag=f"ct{i}", name=f"ct{i}") for i in range(4)]
        for h in range(2):
            for i in range(4):
                tp = psmid.tile([P, P], BF16, tag="m")
                nc.tensor.transpose(
                    tp[:], cent_all[h][:, i * P : (i + 1) * P], identb[:]
                )
                nc.vector.tensor_copy(
                    out=centT[i][:, h * P : (h + 1) * P], in_=tp[:]
                )

        g_ps = psmid.tile([C_LOC, C], F32, tag="m")
        for i in range(4):
            nc.tensor.matmul(
                g_ps[:], clT[i][:], centT[i][:], start=(i == 0), stop=(i == 3)
            )
        g_sb = mid.tile([C_LOC, C], F32, tag="gsb")
        nc.vector.tensor_tensor(out=g_sb[:], in0=g_ps[:], in1=nm_sb[:], op=ALU.add)
        mx = mid.tile([C_LOC, 1], F32, tag="mx")
        nc.vector.reduce_max(mx[:], g_sb[:], axis=AX)
        ns = mid.tile([C_LOC, C], BF16, tag="ns")
        nc.vector.tensor_scalar(
            out=ns[:], in0=g_sb[:], scalar1=mx[:], scalar2=None, op0=ALU.is_equal
        )
        nsT = [mid.tile([P, C_LOC], BF16, tag=f"nsT{h}", name=f"nsT{h}") for h in range(2)]
        for h in range(2):
            tp = psmid.tile([P, C_LOC], BF16, tag="m")
            nc.tensor.transpose(
                tp[:], ns[:, h * P : (h + 1) * P], identb[0:C_LOC, 0:C_LOC]
            )
            nc.vector.tensor_copy(out=nsT[h][:], in_=tp[:])
        cnear_ps = psmid.tile([C_LOC, D], F32, tag="m")
        for h in range(2):
            nc.tensor.matmul(
                cnear_ps[:], nsT[h][:], cent_all[h][:], start=(h == 0), stop=(h == 1)
            )
        u_sb = mid.tile([C_LOC, D], BF16, tag="usb")
        nc.vector.tensor_tensor(
            out=u_sb[:], in0=cnear_ps[:], in1=cent_bf[:], op=ALU.subtract
        )
        # U^T chunks [128d, 32c] for the all-class dot matmuls
        uT = [mid.tile([P, C_LOC], BF16, tag=f"uT{q}", name=f"uT{q}") for q in range(4)]
        for q in range(4):
            tp = psmid.tile([P, C_LOC], BF16, tag="m")
            nc.tensor.transpose(
                tp[:], u_sb[:, q * P : (q + 1) * P], identb[0:C_LOC, 0:C_LOC]
            )
            nc.vector.tensor_copy(out=uT[q][:], in_=tp[:])

        # ---- pass 2: all-class dots on PE, masked extract, reduce ------
        # D^T[c, i] = U_c . e_i computed 8 tiles (1024 samples) per matmul
        # group; dot_i = sum_c ohtT[c, i] * D^T[c, i] via a [32,128] product,
        # a PE transpose, and a [128,32] reduce.
        dot_all = const.tile([P, T], F32)
        con_all = const.tile([P, T], F32)
        GRP = 4

        g0 = 0
        while g0 < T:
            gn = min(GRP, T - g0)
            dps = psu.tile([C_LOC, GRP * P], F32, tag="dps")
            for q in range(4):
                nc.tensor.matmul(
                    dps[:, 0 : gn * P], uT[q][:],
                    eT[q][:, g0 * P : (g0 + gn) * P],
                    start=(q == 0), stop=(q == 3),
                )
            prod = work.tile([C_LOC, GRP * P], BF16, tag="prod")
            nc.vector.tensor_tensor(
                out=prod[:, 0 : gn * P], in0=dps[:, 0 : gn * P],
                in1=ohtT[:, g0 * P : (g0 + gn) * P], op=ALU.mult,
            )
            dotg = psmid.tile([P, GRP], F32, tag="m")
            for j in range(gn):
                nc.tensor.matmul(
                    dotg[:, j : j + 1], prod[:, j * P : (j + 1) * P],
                    ones32[:], start=True, stop=True,
                )
            nc.vector.tensor_copy(
                out=dot_all[:, g0 : g0 + gn], in_=dotg[:, 0:gn]
            )
            # term/weight math for this group, overlapped with PE
            gsl = slice(g0, g0 + gn)
            nc.vector.tensor_tensor(
                out=con_all[:, gsl], in0=dot_all[:, gsl], in1=r_all[:, gsl],
                op=ALU.mult,
            )
            nc.vector.tensor_scalar(
                out=con_all[:, gsl], in0=con_all[:, gsl], scalar1=float(MARGIN),
                scalar2=None, op0=ALU.add,
            )
            nc.scalar.activation(con_all[:, gsl], con_all[:, gsl], ACTF.Relu)
            nc.vector.tensor_tensor(
                out=con_all[:, gsl], in0=con_all[:, gsl], in1=ws_sb[:, gsl],
                op=ALU.mult,
            )
            g0 += GRP

        # ---- loss = sum_i w_i * term_i / n_present (over all cores) ----
        tot_col = mid.tile([P, 1], F32, tag="tot")
        nc.vector.reduce_sum(tot_col[:], con_all[:], axis=AX)
        tot_ps = psmid.tile([1, 1], F32, tag="m")
        nc.tensor.matmul(tot_ps[:], tot_col[:], ones_col[:])
        lloc = mid.tile([1, 1], F32, tag="lloc")
        nc.vector.tensor_copy(out=lloc[:], in_=tot_ps[:])
        tot_sb = mid.tile([1, 8], F32, tag="totsb")
        nc.vector.memset(tot_sb[:], 0.0)
        nc.vector.tensor_copy(out=tot_sb[:, 0:1], in_=lloc[:])
        nc.sync.dma_start(out=ar2_in.ap()[:], in_=tot_sb[:])
        nc.gpsimd.collective_compute(
            "AllReduce", ALU.add, replica_groups=groups,
            ins=[ar2_in.ap()], outs=[ar2_out.ap()],
        )
        nc.sync.dma_start(out=loss_out.ap()[:], in_=ar2_out.ap()[0:1, 0:1])

    nc.compile()
    return nc


_NC = None


def _get_nc():
    global _NC
    if _NC is None:
        _NC = _build()
    return _NC


def build_in_maps(emb: np.ndarray, lab: np.ndarray) -> list[dict]:
    """Class-shard the full batch: core k owns labels [32k, 32k+32)."""
    counts = np.bincount(lab, minlength=C).astype(np.int64)
    order = np.argsort(lab, kind="stable")
    sorted_lab = lab[order]
    bounds = np.searchsorted(sorted_lab, np.arange(0, C + 1, C_LOC))
    n_present = max(int((counts > 0).sum()), 1)
    empty_col = counts == 0  # (C,)

    in_maps = []
    for k in range(N_CORES):
        idx = order[bounds[k] : bounds[k + 1]]
        nk = len(idx)
        assert nk <= B_CAP, f"core {k} got {nk} samples > capacity {B_CAP}"
        emb_k = np.zeros((B_CAP, D), dtype=ml_dtypes.bfloat16)
        emb_k[:nk] = emb[idx].astype(ml_dtypes.bfloat16)
        embT_k = np.ascontiguousarray(emb_k.T)
        lab_k = np.full((B_CAP,), C_LOC, np.int32)
        lab_k[:nk] = lab[idx] - C_LOC * k
        lab_2d = np.ascontiguousarray(lab_k.reshape(T, P).T)  # [P, T]

        nm = np.where(empty_col[None, :], np.float32(NEG), np.float32(0.0))
        nm = np.tile(nm, (C_LOC, 1)).astype(np.float32)
        rows = np.arange(C_LOC)
        nm[rows, C_LOC * k + rows] = NEG  # self-similarity
        w33 = np.zeros(C_LOC + 1, np.float32)
        w33[:C_LOC] = 1.0 / (
            np.maximum(counts[C_LOC * k : C_LOC * (k + 1)], 1) * float(n_present)
        )
        ws_k = np.ascontiguousarray(w33[lab_k].reshape(T, P).T)  # [P, T]
        ohtT_k = (lab_k[None, :] == np.arange(C_LOC)[:, None]).astype(
            ml_dtypes.bfloat16
        )  # [C_LOC, B_CAP]

        in_maps.append(
            {
                "emb": emb_k,
                "embT": embT_k,
                "lab": lab_2d,
                "ohtT": np.ascontiguousarray(ohtT_k),
                "negmask": np.ascontiguousarray(nm),
                "wsamp": ws_k,
            }
        )
    return in_maps


def kernel(embeddings: np.ndarray, labels: np.ndarray) -> np.ndarray:
    emb = np.ascontiguousarray(np.asarray(embeddings, dtype=np.float32))
    lab = np.asarray(labels).astype(np.int32)
    assert emb.shape == (B_FULL, D) and lab.shape == (B_FULL,)

    nc = _get_nc()
    in_maps = build_in_maps(emb, lab)
    res = run_bass_kernel_spmd(nc, in_maps, core_ids=list(range(N_CORES)))
    loss = res.results[0]["loss"]
    return np.asarray(loss, dtype=np.float32).reshape(())


if __name__ == "__main__":
    rng = np.random.default_rng(0)
    e = rng.standard_normal((B_FULL, D), dtype=np.float32)
    l = rng.integers(0, C, size=(B_FULL,)).astype(np.int32)
    print(kernel(embeddings=e, labels=l))



# revision 59
# speedup vs baseline: 1.3961x; 1.3961x over previous
"""Centroid triplet loss on 8 Trainium2 NeuronCores (Bass/Tile).

Class-sharded data parallel: the host assigns core k ALL samples whose label
falls in [32k, 32k+32) (padded to a fixed capacity with zero rows + an
out-of-range label).  Per-class embedding sums are then fully core-local, so
the only centroid communication is a 32KB bf16 AllGather of each core's 32
normalized centroid rows (vs. a 514KB AllReduce for unsorted sharding).

Math (equivalent to the reference):
    term_i = relu(margin + r_i * e_i . (cent[near(l_i)] - cent[l_i]))
    loss   = sum_c (1/count_c) * sum_{i in c} term_i / n_present
Per-sample gathers are replaced by matmuls against the 32-class one-hot:
pass 2 computes u_i = onehot_i . U (U = cent_near - cent_own, [32,512]) with
one PE matmul per 128-sample tile, dots it with e_i on DVE, and reduces the
per-class term sums S_c with tiny [128,1]x[128,32] matmuls.  Label-derived
scalars (1/count, presence mask, 1/n_present) are host-computed inputs.
"""

import sys

for _p in ("/opt/trn_rl_repo",):
    if _p not in sys.path:
        sys.path.insert(0, _p)

from contextlib import ExitStack

import ml_dtypes
import numpy as np

from concourse import bacc, bass, mybir, tile
from concourse.bass_utils import run_bass_kernel_spmd
from concourse.masks import make_identity

F32 = mybir.dt.float32
BF16 = mybir.dt.bfloat16
I32 = mybir.dt.int32
ALU = mybir.AluOpType
ACTF = mybir.ActivationFunctionType
AX = mybir.AxisListType.X

N_CORES = 8
B_FULL = 65536
D = 512
C = 256
C_LOC = C // N_CORES        # 32 classes owned per core
MARGIN = 0.3
EPS = 1e-12
NEG = -1e30

P = 128                      # SBUF partitions
B_CAP = 8704                 # padded per-core sample capacity (mean 8192)
T = B_CAP // P               # 68 sample tiles of 128
NCHUNK = 17                  # tiles per norm batch / load DMA
WARMUP_AR = True             # dummy tiny AllReduce to absorb CC bootstrap


def _build():
    nc = bacc.Bacc(
        "TRN2",
        target_bir_lowering=False,
        debug=False,
        enable_asserts=False,
        num_devices=N_CORES,
    )

    emb = nc.dram_tensor("emb", [B_CAP, D], BF16, kind="ExternalInput")
    lab = nc.dram_tensor("lab", [P, T], I32, kind="ExternalInput")
    ohtT_d = nc.dram_tensor("ohtT", [C_LOC, B_CAP], BF16, kind="ExternalInput")
    embT_d = nc.dram_tensor("embT", [D, B_CAP], BF16, kind="ExternalInput")
    negmask = nc.dram_tensor("negmask", [C_LOC, C], F32, kind="ExternalInput")
    wsamp = nc.dram_tensor("wsamp", [P, T], F32, kind="ExternalInput")
    loss_out = nc.dram_tensor("loss", [1, 1], F32, kind="ExternalOutput")

    ag_in = nc.dram_tensor("ag_in", [C_LOC, D], BF16)
    ag_out = nc.dram_tensor("ag_out", [C, D], BF16, addr_space="Shared")
    ar2_in = nc.dram_tensor("ar2_in", [1, 8], F32)
    ar2_out = nc.dram_tensor("ar2_out", [1, 8], F32, addr_space="Shared")
    if WARMUP_AR:
        ar0_in = nc.dram_tensor("ar0_in", [1, 8], F32)
        ar0_out = nc.dram_tensor("ar0_out", [1, 8], F32, addr_space="Shared")

    groups = [list(range(N_CORES))]

    with tile.TileContext(nc) as tc, ExitStack() as ctx:
        const = ctx.enter_context(tc.tile_pool(name="const", bufs=1))
        big = ctx.enter_context(tc.tile_pool(name="big", bufs=1))
        work = ctx.enter_context(tc.tile_pool(name="work", bufs=3))
        sq = ctx.enter_context(tc.tile_pool(name="sq", bufs=3))
        mid = ctx.enter_context(tc.tile_pool(name="mid", bufs=1))
        psacc = ctx.enter_context(tc.tile_pool(name="psacc", bufs=1, space="PSUM"))
        psmid = ctx.enter_context(tc.tile_pool(name="psmid", bufs=3, space="PSUM"))
        psu = ctx.enter_context(tc.tile_pool(name="psu", bufs=3, space="PSUM"))

        # ---- warm up the collective stream under the load DMA ----------
        if WARMUP_AR:
            ar0_sb = mid.tile([1, 8], F32, tag="ar0")
            nc.vector.memset(ar0_sb[:], 0.0)
            nc.sync.dma_start(out=ar0_in.ap()[:], in_=ar0_sb[:])
            nc.gpsimd.collective_compute(
                "AllReduce", ALU.add, replica_groups=groups,
                ins=[ar0_in.ap()], outs=[ar0_out.ap()],
            )

        # ---- constants -------------------------------------------------
        ident = const.tile([P, P], F32)
        make_identity(nc, ident[:])
        identb = const.tile([P, P], BF16)
        nc.vector.tensor_copy(out=identb[:], in_=ident[:])
        iota_row = const.tile([P, C_LOC], BF16)
        nc.gpsimd.iota(
            iota_row[:], pattern=[[1, C_LOC]], base=0, channel_multiplier=0,
            allow_small_or_imprecise_dtypes=True,
        )
        ones_col = const.tile([P, 1], F32)
        nc.gpsimd.memset(ones_col[:], 1.0)
        ones32 = const.tile([C_LOC, 1], BF16)
        nc.gpsimd.memset(ones32[:], 1.0)

        lab_sb = const.tile([P, T], I32)
        nc.sync.dma_start(out=lab_sb[:], in_=lab.ap())
        lab_f = const.tile([P, T], BF16)
        nc.vector.tensor_copy(out=lab_f[:], in_=lab_sb[:])
        # ---- pass 1: load embeddings, norms, local class sums ----------
        chunks = []
        t0 = 0
        while t0 < T:
            chunks.append((t0, min(NCHUNK, T - t0)))
            t0 += NCHUNK

        e_chunks = {}
        emb_v = emb.ap().rearrange("(t p) d -> p t d", p=P)
        for (c0, cn) in chunks:
            ec = big.tile([P, cn, D], BF16, tag=f"e{c0}")
            e_chunks[c0] = ec
            nc.sync.dma_start(out=ec[:], in_=emb_v[:, c0 : c0 + cn, :])

        # pass-2-only data loads go behind the embedding chunks
        nm_sb = const.tile([C_LOC, C], F32)
        nc.sync.dma_start(out=nm_sb[:], in_=negmask.ap())
        ws_sb = const.tile([P, T], F32)
        nc.sync.dma_start(out=ws_sb[:], in_=wsamp.ap())
        ohtT = const.tile([C_LOC, T * P], BF16)
        nc.sync.dma_start(out=ohtT[:], in_=ohtT_d.ap())
        eT = [
            const.tile([P, B_CAP], BF16, tag=f"eT{q}", name=f"eT{q}")
            for q in range(4)
        ]
        for q in range(4):
            nc.sync.dma_start(out=eT[q][:], in_=embT_d.ap()[q * P : (q + 1) * P, :])

        def e_tile(t):
            c0 = (t // NCHUNK) * NCHUNK
            return e_chunks[c0][:, t - c0, :]

        norm2 = const.tile([P, T], F32)
        r_all = const.tile([P, T], F32)
        r_bf = const.tile([P, T], BF16)

        sums_ps = psacc.tile([C_LOC, D], F32, tag="sums")

        for (c0, cn) in chunks:
            csl = slice(c0, c0 + cn)
            for j in range(cn):
                t = c0 + j
                et = e_tile(t)
                if t % 3 != 2:
                    sq_t = sq.tile([P, D], F32, tag="sq")
                    nc.scalar.activation(
                        sq_t[:], et, ACTF.Square, accum_out=norm2[:, t : t + 1]
                    )
                else:
                    pr_t = sq.tile([P, D], BF16, tag="pr")
                    nc.gpsimd.tensor_tensor(out=pr_t[:], in0=et, in1=et, op=ALU.mult)
                    nc.vector.reduce_sum(norm2[:, t : t + 1], pr_t[:], axis=AX)
            # batched norm -> r for the chunk (clamped so zero pads stay finite)
            nc.scalar.activation(r_all[:, csl], norm2[:, csl], ACTF.Sqrt)
            nc.vector.tensor_scalar(
                out=r_all[:, csl], in0=r_all[:, csl], scalar1=EPS, scalar2=None,
                op0=ALU.max,
            )
            nc.vector.reciprocal(r_all[:, csl], r_all[:, csl])
            nc.vector.tensor_copy(out=r_bf[:, csl], in_=r_all[:, csl])

            for j in range(cn):
                t = c0 + j
                oht_t = work.tile([P, C_LOC], BF16, tag="oht")
                nc.vector.tensor_tensor(
                    out=oht_t[:], in0=iota_row[:],
                    in1=lab_f[:, t : t + 1].to_broadcast([P, C_LOC]),
                    op=ALU.is_equal,
                )
                osc = work.tile([P, C_LOC], BF16, tag="osc")
                nc.vector.tensor_tensor(
                    out=osc[:], in0=oht_t[:],
                    in1=r_bf[:, t : t + 1].to_broadcast([P, C_LOC]),
                    op=ALU.mult,
                )
                nc.tensor.matmul(
                    sums_ps[:], osc[:], e_tile(t),
                    start=(t == 0), stop=(t == T - 1),
                )

        # ---- local centroids + AllGather -------------------------------
        sums_sb = mid.tile([C_LOC, D], F32, tag="ssb")
        nc.vector.tensor_copy(out=sums_sb[:], in_=sums_ps[:])
        s2 = sq.tile([C_LOC, D], F32, tag="sq")
        cn2 = mid.tile([C_LOC, 1], F32, tag="cn2")
        nc.scalar.activation(s2[:], sums_ps[:], ACTF.Square, accum_out=cn2[:])
        nc.scalar.activation(cn2[:], cn2[:], ACTF.Sqrt)
        nc.vector.tensor_scalar(
            out=cn2[:], in0=cn2[:], scalar1=EPS, scalar2=None, op0=ALU.max
        )
        nc.vector.reciprocal(cn2[:], cn2[:])
        cent_bf = mid.tile([C_LOC, D], BF16, tag="centbf")
        nc.vector.tensor_scalar(
            out=cent_bf[:], in0=sums_sb[:], scalar1=cn2[:], scalar2=None,
            op0=ALU.mult,
        )
        nc.sync.dma_start(out=ag_in.ap()[:], in_=cent_bf[:])
        nc.gpsimd.collective_compute(
            "AllGather", ALU.bypass, replica_groups=groups,
            ins=[ag_in.ap()], outs=[ag_out.ap()],
        )

        # local centroid transpose (f32 -> bf16 chunks) for the G matmul
        clT = [mid.tile([P, C_LOC], BF16, tag=f"clT{i}", name=f"clT{i}") for i in range(4)]
        for i in range(4):
            tp = psmid.tile([P, C_LOC], BF16, tag="m")
            nc.tensor.transpose(
                tp[:], cent_bf[:, i * P : (i + 1) * P], identb[0:C_LOC, 0:C_LOC]
            )
            nc.vector.tensor_copy(out=clT[i][:], in_=tp[:])

        # ---- gathered centroids; G rows; nearest; U --------------------
        cent_all = [mid.tile([P, D], BF16, tag=f"ca{h}", name=f"ca{h}") for h in range(2)]
        for h in range(2):
            nc.sync.dma_start(out=cent_all[h][:], in_=ag_out.ap()[h * P : (h + 1) * P, :])
        centT = [mid.tile([P, C], BF16, tag=f"ct{i}", name=f"ct{i}") for i in range(4)]
        for h in range(2):
            for i in range(4):
                tp = psmid.tile([P, P], BF16, tag="m")
                nc.tensor.transpose(
                    tp[:], cent_all[h][:, i * P : (i + 1) * P], identb[:]
                )
                nc.vector.tensor_copy(
                    out=centT[i][:, h * P : (h + 1) * P], in_=tp[:]
                )

        g_ps = psmid.tile([C_LOC, C], F32, tag="m")
        for i in range(4):
            nc.tensor.matmul(
                g_ps[:], clT[i][:], centT[i][:], start=(i == 0), stop=(i == 3)
            )
        g_sb = mid.tile([C_LOC, C], F32, tag="gsb")
        nc.vector.tensor_tensor(out=g_sb[:], in0=g_ps[:], in1=nm_sb[:], op=ALU.add)
        mx = mid.tile([C_LOC, 1], F32, tag="mx")
        nc.vector.reduce_max(mx[:], g_sb[:], axis=AX)
        ns = mid.tile([C_LOC, C], BF16, tag="ns")
        nc.vector.tensor_scalar(
            out=ns[:], in0=g_sb[:], scalar1=mx[:], scalar2=None, op0=ALU.is_equal
        )
        nsT = [mid.tile([P, C_LOC], BF16, tag=f"nsT{h}", name=f"nsT{h}") for h in range(2)]
        for h in range(2):
            tp = psmid.tile([P, C_LOC], BF16, tag="m")
            nc.tensor.transpose(
                tp[:], ns[:, h * P : (h + 1) * P], identb[0:C_LOC, 0:C_LOC]
            )
            nc.vector.tensor_copy(out=nsT[h][:], in_=tp[:])
        cnear_ps = psmid.tile([C_LOC, D], F32, tag="m")
        for h in range(2):
            nc.tensor.matmul(
                cnear_ps[:], nsT[h][:], cent_all[h][:], start=(h == 0), stop=(h == 1)
            )
        u_sb = mid.tile([C_LOC, D], BF16, tag="usb")
        nc.vector.tensor_tensor(
            out=u_sb[:], in0=cnear_ps[:], in1=cent_bf[:], op=ALU.subtract
        )
        # U^T chunks [128d, 32c] for the all-class dot matmuls
        uT = [mid.tile([P, C_LOC], BF16, tag=f"uT{q}", name=f"uT{q}") for q in range(4)]
        for q in range(4):
            tp = psmid.tile([P, C_LOC], BF16, tag="m")
            nc.tensor.transpose(
                tp[:], u_sb[:, q * P : (q + 1) * P], identb[0:C_LOC, 0:C_LOC]
            )
            nc.vector.tensor_copy(out=uT[q][:], in_=tp[:])

        # ---- pass 2: all-class dots on PE, masked extract, reduce ------
        # D^T[c, i] = U_c . e_i computed 8 tiles (1024 samples) per matmul
        # group; dot_i = sum_c ohtT[c, i] * D^T[c, i] via a [32,128] product,
        # a PE transpose, and a [128,32] reduce.
        dot_all = const.tile([P, T], F32)
        con_all = const.tile([P, T], F32)
        GRP = 4

        g0 = 0
        while g0 < T:
            gn = min(GRP, T - g0)
            dps = psu.tile([C_LOC, GRP * P], F32, tag="dps")
            for q in range(4):
                nc.tensor.matmul(
                    dps[:, 0 : gn * P], uT[q][:],
                    eT[q][:, g0 * P : (g0 + gn) * P],
                    start=(q == 0), stop=(q == 3),
                )
            prod = work.tile([C_LOC, GRP * P], BF16, tag="prod")
            nc.vector.tensor_tensor(
                out=prod[:, 0 : gn * P], in0=dps[:, 0 : gn * P],
                in1=ohtT[:, g0 * P : (g0 + gn) * P], op=ALU.mult,
            )
            dotg = psmid.tile([P, GRP], F32, tag="m")
            for j in range(gn):
                nc.tensor.matmul(
                    dotg[:, j : j + 1], prod[:, j * P : (j + 1) * P],
                    ones32[:], start=True, stop=True,
                )
            nc.vector.tensor_copy(
                out=dot_all[:, g0 : g0 + gn], in_=dotg[:, 0:gn]
            )
            # term/weight math for this group, overlapped with PE
            gsl = slice(g0, g0 + gn)
            nc.vector.tensor_tensor(
                out=con_all[:, gsl], in0=dot_all[:, gsl], in1=r_all[:, gsl],
                op=ALU.mult,
            )
            nc.vector.tensor_scalar(
                out=con_all[:, gsl], in0=con_all[:, gsl], scalar1=float(MARGIN),
                scalar2=None, op0=ALU.add,
            )
            nc.scalar.activation(con_all[:, gsl], con_all[:, gsl], ACTF.Relu)
            nc.vector.tensor_tensor(
                out=con_all[:, gsl], in0=con_all[:, gsl], in1=ws_sb[:, gsl],
                op=ALU.mult,
            )
            g0 += GRP

        # ---- loss = sum_i w_i * term_i / n_present (over all cores) ----
        tot_col = mid.tile([P, 1], F32, tag="tot")
        nc.vector.reduce_sum(tot_col[:], con_all[:], axis=AX)
        tot_ps = psmid.tile([1, 1], F32, tag="m")
        nc.tensor.matmul(tot_ps[:], tot_col[:], ones_col[:])
        lloc = mid.tile([1, 1], F32, tag="lloc")
        nc.vector.tensor_copy(out=lloc[:], in_=tot_ps[:])
        tot_sb = mid.tile([1, 8], F32, tag="totsb")
        nc.vector.memset(tot_sb[:], 0.0)
        nc.vector.tensor_copy(out=tot_sb[:, 0:1], in_=lloc[:])
        nc.sync.dma_start(out=ar2_in.ap()[:], in_=tot_sb[:])
        nc.gpsimd.collective_compute(
            "AllReduce", ALU.add, replica_groups=groups,
            ins=[ar2_in.ap()], outs=[ar2_out.ap()],
        )
        nc.sync.dma_start(out=loss_out.ap()[:], in_=ar2_out.ap()[0:1, 0:1])

    nc.compile()
    return nc


_NC = None


def _get_nc():
    global _NC
    if _NC is None:
        _NC = _build()
    return _NC


def build_in_maps(emb: np.ndarray, lab: np.ndarray) -> list[dict]:
    """Class-shard the full batch: core k owns labels [32k, 32k+32)."""
    counts = np.bincount(lab, minlength=C).astype(np.int64)
    order = np.argsort(lab, kind="stable")
    sorted_lab = lab[order]
    bounds = np.searchsorted(sorted_lab, np.arange(0, C + 1, C_LOC))
    n_present = max(int((counts > 0).sum()), 1)
    empty_col = counts == 0  # (C,)

    in_maps = []
    for k in range(N_CORES):
        idx = order[bounds[k] : bounds[k + 1]]
        nk = len(idx)
        assert nk <= B_CAP, f"core {k} got {nk} samples > capacity {B_CAP}"
        emb_k = np.zeros((B_CAP, D), dtype=ml_dtypes.bfloat16)
        emb_k[:nk] = emb[idx].astype(ml_dtypes.bfloat16)
        embT_k = np.ascontiguousarray(emb_k.T)
        lab_k = np.full((B_CAP,), C_LOC, np.int32)
        lab_k[:nk] = lab[idx] - C_LOC * k
        lab_2d = np.ascontiguousarray(lab_k.reshape(T, P).T)  # [P, T]

        nm = np.where(empty_col[None, :], np.float32(NEG), np.float32(0.0))
        nm = np.tile(nm, (C_LOC, 1)).astype(np.float32)
        rows = np.arange(C_LOC)
        nm[rows, C_LOC * k + rows] = NEG  # self-similarity
        w33 = np.zeros(C_LOC + 1, np.float32)
        w33[:C_LOC] = 1.0 / (
            np.maximum(counts[C_LOC * k : C_LOC * (k + 1)], 1) * float(n_present)
        )
        ws_k = np.ascontiguousarray(w33[lab_k].reshape(T, P).T)  # [P, T]
        ohtT_k = (lab_k[None, :] == np.arange(C_LOC)[:, None]).astype(
            ml_dtypes.bfloat16
        )  # [C_LOC, B_CAP]

        in_maps.append(
            {
                "emb": emb_k,
                "embT": embT_k,
                "lab": lab_2d,
                "ohtT": np.ascontiguousarray(ohtT_k),
                "negmask": np.ascontiguousarray(nm),
                "wsamp": ws_k,
            }
        )
    return in_maps


def kernel(embeddings: np.ndarray, labels: np.ndarray) -> np.ndarray:
    emb = np.ascontiguousarray(np.asarray(embeddings, dtype=np.float32))
    lab = np.asarray(labels).astype(np.int32)
    assert emb.shape == (B_FULL, D) and lab.shape == (B_FULL,)

    nc = _get_nc()
    in_maps = build_in_maps(emb, lab)
    res = run_bass_kernel_spmd(nc, in_maps, core_ids=list(range(N_CORES)))
    loss = res.results[0]["loss"]
    return np.asarray(loss, dtype=np.float32).reshape(())


if __name__ == "__main__":
    rng = np.random.default_rng(0)
    e = rng.standard_normal((B_FULL, D), dtype=np.float32)
    l = rng.integers(0, C, size=(B_FULL,)).astype(np.int32)
    print(kernel(embeddings=e, labels=l))

